# revision 1
# baseline (speedup 1.0000x reference)
"""Distributed self-attention kernel for Trainium2, 8 NeuronCores.

Sequence-parallel (the module's own sharding): S=3072 is sharded 384
rows/core. Each core computes its q/k/v projection chunk in bf16 (fp32
PSUM accumulation); two AllGathers (v first, then k^T — each ~0.77
MB/rank in bf16) share K/V, then each core runs attention for its 384
queries over all 3072 keys and writes its slice of the output.

Attention runs in transposed-score layout (s^T[key, query]) so P@V needs
no transpose of P. All matmul stationaries are full 128x128 (partial
stationaries stream at half rate on TRN2):
  - scores: stationary = k^T pair-block [128 dims, 128 keys]; the query
    rhs is zero-padded per head ([q_even; 0] / [0; q_odd]) so each head's
    scores come out exact at full rate.
  - P@V: stationary = the contiguous 128-col window [v_h | ones | ...]
    of a 65-interleaved v tile; output rows 0-63 are context, row 64
    accumulates the softmax denominator, rows 65-127 catch garbage from
    the next head's v and are never read.
exp() on ScalarE with the 1/sqrt(64) scale fused (no max subtraction:
softmax logits here are |qk/8| < ~4, exp is safely in fp32/bf16 range —
mathematically identical to the reference's max-subtracted softmax).
Even/odd heads are staggered so ScalarE exp and PE matmuls overlap within
exactly 8 PSUM banks. PE-transpose + VectorE normalize finish each head;
the core DMAs out its [384, 1024] slice and the host concatenates.
"""

import numpy as np
import ml_dtypes

import concourse.bacc as bacc
import concourse.mybir as mybir
import concourse.tile as tile
from concourse import bass_utils

F32 = mybir.dt.float32
BF16 = mybir.dt.bfloat16
AF = mybir.ActivationFunctionType

N_CORES = 8
B, S, HID = 1, 3072, 1024
NH, HD = 16, 64
SC = S // N_CORES          # 384 sequence rows per core
QT = SC // 128             # 3 query sub-tiles per core
KT = S // 128              # 24 key tiles globally
KIN = HID + 1              # augmented contraction (bias row)
NG = KT // 3               # 8 groups of 3 key tiles for batched exp
VW = NH * 65               # interleaved v width: [v_h (64) | 1.0] x 16 = 1040
VTW = VW + 63              # v tile width incl. zero tail so head 15's
                           # 128-col stationary window stays in bounds

_KSZ = 8 * 128 * SC        # k^T elements per rank in the AG payload
_VSZ = QT * 128 * VW       # v elements per rank (65-interleaved layout)
_PAYLOAD = _KSZ + _VSZ

_cache: dict = {}


def _build(with_mask: bool):
    nc = bacc.Bacc("TRN2", target_bir_lowering=False, debug=False,
                   num_devices=N_CORES)

    xt = nc.dram_tensor("xt", [KIN, SC], BF16, kind="ExternalInput")
    w = nc.dram_tensor("w", [3, KIN, HID], BF16, kind="ExternalInput")
    ident = nc.dram_tensor("ident", [128, 128], F32, kind="ExternalInput")
    if with_mask:
        maskt = nc.dram_tensor("maskt", [128, KT], F32, kind="ExternalInput")
    out = nc.dram_tensor("out", [SC, HID], F32, kind="ExternalOutput")

    with tile.TileContext(nc) as tc:
        with (
            tc.tile_pool(name="persist", bufs=1) as pp,
            tc.tile_pool(name="dram", bufs=1, space="DRAM") as dram,
        ):
            kin = dram.tile([_KSZ], BF16)
            kout = dram.tile([N_CORES, _KSZ], BF16, addr_space="Shared")
            vin = dram.tile([_VSZ], BF16)
            vout = dram.tile([N_CORES, _VSZ], BF16, addr_space="Shared")

            # ---- persistent SBUF tensors ----
            xsb = pp.tile([128, 9 * SC], BF16)       # x^T, 9 contraction slices
            qz = [pp.tile([128, 2 * SC], BF16, name=f"qz{m}") for m in range(8)]
            ksb = [pp.tile([128, S], BF16, name=f"ksb{h}") for h in range(8)]
            vsb = [pp.tile([128, VTW], BF16, name=f"vsb{k}") for k in range(KT)]
            idsb = pp.tile([128, 128], F32)
            ctxsb = [pp.tile([65, SC], F32, name=f"ctxsb{h}") for h in range(NH)]
            osb = [pp.tile([128, HID], F32, name=f"osb{t}") for t in range(QT)]
            if with_mask:
                msb = pp.tile([128, KT], F32)
                nc.sync.dma_start(msb[:], maskt[:])

            # x^T load: 8 full slices + 1-row bias slice
            for j in range(8):
                nc.sync.dma_start(xsb[:, j * SC:(j + 1) * SC],
                                  xt[j * 128:(j + 1) * 128, :])
            nc.sync.dma_start(xsb[0:1, 8 * SC:9 * SC], xt[1024:1025, :])

            # zero-pad q pair tiles (must precede the q-projection copies)
            for m in range(8):
                nc.vector.memset(qz[m][:], 0.0)

            # ---- phase A: projections ----
            with (
                tc.tile_pool(name="wpool", bufs=6) as wpool,
                tc.tile_pool(name="stg", bufs=4) as stg,
                tc.tile_pool(name="ppsum", bufs=4, space="PSUM") as ppsum,
            ):
                def load_w(proj, j):
                    if j < 8:
                        wt = wpool.tile([128, HID], BF16, tag="w", bufs=12,
                                        name=f"w{proj}_{j}")
                        nc.sync.dma_start(wt[:], w[proj, j * 128:(j + 1) * 128, :])
                    else:
                        wt = wpool.tile([1, HID], BF16, tag="wb", bufs=3,
                                        name=f"wb{proj}")
                        nc.sync.dma_start(wt[:], w[proj, HID:HID + 1, :])
                    return wt

                # v first: every pair's PV walks all of V from its
                # first group, while k^T is consumed progressively per pair
                # — so the k AllGather can finish second without stalling.
                wv = [load_w(2, j) for j in range(9)]
                for st in range(QT):
                    # staging tile already 65-interleaved with the ones
                    # columns, so post-AG v loads are fully contiguous
                    vst = stg.tile([128, VW], BF16, tag="vst", name=f"vst{st}")
                    vst3 = vst.rearrange("p (h y) -> p h y", y=65)
                    nc.vector.memset(vst3[:, :, 64:65], 1.0)
                    for half in range(2):
                        pv = ppsum.tile([128, 512], F32, tag="pv", bufs=3,
                                        name=f"pv{st}_{half}")
                        for j in range(9):
                            rows = 128 if j < 8 else 1
                            nc.tensor.matmul(
                                pv[:],
                                xsb[:rows, j * SC + st * 128: j * SC + (st + 1) * 128],
                                wv[j][:rows, half * 512:(half + 1) * 512],
                                start=(j == 0), stop=(j == 8))
                        nc.vector.tensor_copy(
                            vst3[:, half * 8:(half + 1) * 8, 0:64],
                            pv[:].rearrange("p (h y) -> p h y", y=HD))
                    nc.sync.dma_start(
                        vin[st * 128 * VW:(st + 1) * 128 * VW]
                        .rearrange("(p x) -> p x", x=VW), vst[:])

                nc.gpsimd.collective_compute(
                    "AllGather",
                    mybir.AluOpType.bypass,
                    replica_groups=[list(range(N_CORES))],
                    ins=[vin[:].opt()],
                    outs=[vout[:].opt()],
                )

                wk = [load_w(1, j) for j in range(9)]
                for m in range(8):
                    pk = ppsum.tile([128, SC], F32, tag="pk", bufs=5, name=f"pk{m}")
                    for j in range(9):
                        rows = 128 if j < 8 else 1
                        nc.tensor.matmul(
                            pk[:], wk[j][:rows, m * 128:(m + 1) * 128],
                            xsb[:rows, j * SC:(j + 1) * SC],
                            start=(j == 0), stop=(j == 8))
                    kst = stg.tile([128, SC], BF16, tag="kst", name=f"kst{m}")
                    nc.vector.tensor_copy(kst[:], pk[:])
                    nc.sync.dma_start(
                        kin[m * 128 * SC:(m + 1) * 128 * SC]
                        .rearrange("(p x) -> p x", x=SC), kst[:])

                nc.gpsimd.collective_compute(
                    "AllGather",
                    mybir.AluOpType.bypass,
                    replica_groups=[list(range(N_CORES))],
                    ins=[kin[:].opt()],
                    outs=[kout[:].opt()],
                )

                wq = [load_w(0, j) for j in range(9)]
                for m in range(8):
                    pq = ppsum.tile([128, SC], F32, tag="pk", bufs=5, name=f"pq{m}")
                    for j in range(9):
                        rows = 128 if j < 8 else 1
                        nc.tensor.matmul(
                            pq[:], wq[j][:rows, m * 128:(m + 1) * 128],
                            xsb[:rows, j * SC:(j + 1) * SC],
                            start=(j == 0), stop=(j == 8))
                    # zero-padded halves: head-even in cols 0:SC (rows 0-63),
                    # head-odd in cols SC:2SC (rows 64-127)
                    nc.vector.tensor_copy(qz[m][0:64, 0:SC], pq[0:64, :])
                    nc.vector.tensor_copy(qz[m][64:128, SC:2 * SC], pq[64:128, :])

            # late prologue pieces, needed only by phase C/D — emitted
            # after the projection/AllGather chain so they don't compete
            nc.sync.dma_start(idsb[:], ident[:])
            for k in range(KT):
                nc.vector.memset(vsb[k][:, VW:VTW], 0.0)

            # ---- phase B: spread gathered K/V into SBUF ----
            # ALL v loads are emitted before any k load: the SP sequencer is
            # a FIFO, and the v AllGather finishes ~35us before the k one —
            # v triggers parked behind a k-wait would burn that head start.
            # k triggers can't fire before their AllGather lands anyway.
            for k in range(KT):
                r, st = k // QT, k % QT
                nc.sync.dma_start(
                    vsb[k][:, 0:VW],
                    vout[r, st * 128 * VW:(st + 1) * 128 * VW]
                    .rearrange("(p x) -> p x", x=VW))
            for hp in range(8):
                for r in range(N_CORES):
                    nc.sync.dma_start(
                        ksb[hp][:, r * SC:(r + 1) * SC],
                        kout[r, hp * 128 * SC:(hp + 1) * 128 * SC]
                        .rearrange("(p x) -> p x", x=SC))

            # ---- phase C: attention, staggered even/odd heads ----
            with (
                tc.tile_pool(name="spoolE", bufs=1, space="PSUM") as spoolE,
                tc.tile_pool(name="spoolO", bufs=1, space="PSUM") as spoolO,
                tc.tile_pool(name="cpool", bufs=1, space="PSUM") as cpool,
                tc.tile_pool(name="ppool", bufs=10) as ppool,
            ):
                def score_block(sp, hp, e, g):
                    for j in range(3):
                        kt = g * 3 + j
                        nc.tensor.matmul(
                            sp[:, j * 512: j * 512 + SC],
                            ksb[hp][:, kt * 128:(kt + 1) * 128],
                            qz[hp][:, e * SC:(e + 1) * SC],
                            start=True, stop=True)

                def exp_block(pt, sp, g):
                    src3 = sp.rearrange("p (g x) -> p g x", x=512)[:, :, 0:SC]
                    dst3 = pt.rearrange("p (g x) -> p g x", x=SC)
                    if with_mask:
                        for j in range(3):
                            kt = g * 3 + j
                            nc.scalar.activation(
                                dst3[:, j, :], src3[:, j, :], AF.Exp,
                                bias=msb[:, kt:kt + 1], scale=0.125)
                    else:
                        nc.scalar.activation(dst3, src3, AF.Exp, scale=0.125)

                def pv_block(ctx, pt, h, g):
                    # stationary = contiguous [v_h | ones | v_{h+1}...] window;
                    # out rows 0-63 = ctx, row 64 = denominator, rows 65-127
                    # accumulate next-head garbage that is never read.
                    for j in range(3):
                        kt = g * 3 + j
                        nc.tensor.matmul(
                            ctx[:], vsb[kt][:, 65 * h: 65 * h + 128],
                            pt[:, j * SC:(j + 1) * SC],
                            start=(g == 0 and j == 0),
                            stop=(g == NG - 1 and j == 2))

                for hp in range(8):
                    ctxE = cpool.tile([128, SC], F32, tag="ctxE", name=f"cE{hp}")
                    ctxO = cpool.tile([128, SC], F32, tag="ctxO", name=f"cO{hp}")
                    for g in range(NG):
                        spE = spoolE.tile([128, 1536], F32, tag="spE",
                                          name=f"spE{hp}_{g}")
                        score_block(spE, hp, 0, g)
                        ptE = ppool.tile([128, 3 * SC], BF16, tag="pt",
                                         name=f"ptE{hp}_{g}")
                        exp_block(ptE, spE, g)
                        spO = spoolO.tile([128, 1536], F32, tag="spO",
                                          name=f"spO{hp}_{g}")
                        score_block(spO, hp, 1, g)
                        ptO = ppool.tile([128, 3 * SC], BF16, tag="pt",
                                         name=f"ptO{hp}_{g}")
                        exp_block(ptO, spO, g)
                        pv_block(ctxE, ptE, 2 * hp, g)
                        pv_block(ctxO, ptO, 2 * hp + 1, g)
                    nc.vector.tensor_copy(ctxsb[2 * hp][:], ctxE[0:65, :])
                    nc.vector.tensor_copy(ctxsb[2 * hp + 1][:], ctxO[0:65, :])

            # ---- phase D: transpose back, normalize, store ----
            with (
                tc.tile_pool(name="tpool", bufs=8, space="PSUM") as tpool,
                tc.tile_pool(name="rpool2", bufs=8) as rpool2,
            ):
                for h in range(NH):
                    for t in range(QT):
                        tp = tpool.tile([128, 65], F32, tag="tp",
                                        name=f"tp{h}_{t}")
                        nc.tensor.transpose(
                            tp[:], ctxsb[h][:, t * 128:(t + 1) * 128],
                            idsb[0:65, 0:65])
                        rec = rpool2.tile([128, 1], F32, tag="rec",
                                          name=f"rec{h}_{t}")
                        nc.vector.reciprocal(rec[:], tp[:, 64:65])
                        nc.vector.tensor_scalar_mul(
                            osb[t][:, h * HD:(h + 1) * HD], tp[:, 0:64], rec[:])
                for t in range(QT):
                    nc.sync.dma_start(out[t * 128:(t + 1) * 128, :], osb[t][:])

    nc.compile()
    return nc


def _get_program(with_mask: bool):
    key = ("prog", with_mask)
    if key not in _cache:
        _cache[key] = _build(with_mask)
    return _cache[key]


def kernel(hidden_states, attention_mask, Wq, bq, Wk, bk, Wv, bv):
    x = np.asarray(hidden_states, np.float32).reshape(S, HID)
    mask = np.asarray(attention_mask, np.float32).reshape(-1)
    if mask.size == 1:
        mask = np.full(S, float(mask[0]), np.float32)
    with_mask = bool(np.any(mask))

    # augmented weights: [3, 1025, 1024] with the bias as the last
    # contraction row; x^T gets a matching ones row.
    w_aug = np.empty((3, KIN, HID), np.float32)
    for i, (W, b) in enumerate(((Wq, bq), (Wk, bk), (Wv, bv))):
        w_aug[i, :HID] = np.asarray(W, np.float32).T
        w_aug[i, HID] = np.asarray(b, np.float32)
    w_aug = w_aug.astype(ml_dtypes.bfloat16)

    ident = np.eye(128, dtype=np.float32)

    nc = _get_program(with_mask)
    in_maps = []
    for c in range(N_CORES):
        xtc = np.empty((KIN, SC), np.float32)
        xtc[:HID] = x[c * SC:(c + 1) * SC, :].T
        xtc[HID] = 1.0
        m = {
            "xt": xtc.astype(ml_dtypes.bfloat16),
            "w": w_aug,
            "ident": ident,
        }
        if with_mask:
            m["maskt"] = np.ascontiguousarray(
                mask.reshape(KT, 128).T.astype(np.float32))
        in_maps.append(m)

    _cache["last_in_maps"] = in_maps
    res = bass_utils.run_bass_kernel_spmd(nc, in_maps, core_ids=list(range(N_CORES)))
    out = np.concatenate([res.results[c]["out"] for c in range(N_CORES)], axis=0)
    return out.reshape(B, S, HID).astype(np.float32)



# revision 6
# speedup vs baseline: 1.4399x; 1.4399x over previous
"""Distributed self-attention kernel for Trainium2, 8 NeuronCores.

Head-parallel sharding (no collectives): with NH=16 heads on 8 cores,
each core owns one head PAIR (heads 2c, 2c+1 = hidden dims 128c..128c+128).
Every core loads the full x^T (6.3 MB bf16 — ~18us of DMA, far cheaper
than the ~100us the sequence-parallel AllGathers cost on this fabric),
computes q/k/v projections restricted to its pair's 128 output dims over
the whole sequence, runs attention for 2 heads x 3072 queries, and writes
its [3072, 128] slice of the hidden dim; the host concatenates.

Attention runs in transposed-score layout (s^T[key, query]) so P@V needs
no transpose of P. All matmul stationaries are full 128x128:
  - scores: stationary = k^T pair-block [128 dims, 128 keys]; the query
    rhs is zero-padded per head ([q_even; 0] / [0; q_odd]) so each head's
    scores come out exact at full rate.
  - P@V: stationary = a 128-col window of the per-key-tile v layout
    [v_even(64) | 1 | v_odd(64) | 1 | 0-pad(63)]; output rows 0-63 are
    context, row 64 accumulates the softmax denominator, rows 65-127
    catch garbage from the other head's v and are never read.
exp() on ScalarE with the 1/sqrt(64) scale fused (no max subtraction:
softmax logits here are |qk/8| < ~4, exp is safely in fp32/bf16 range —
mathematically identical to the reference's max-subtracted softmax).
Even/odd heads are staggered so ScalarE exp and PE matmuls overlap within
exactly 8 PSUM banks (spE 3 + spO 3 + ctxE 1 + ctxO 1). Queries are
processed in 6 chunks of 512 so every matmul streams 512 moving columns.
PE-transpose + VectorE normalize finish each head; the core DMAs out its
[3072, 128] slice.
"""

import numpy as np
import ml_dtypes

import concourse.bacc as bacc
import concourse.mybir as mybir
import concourse.tile as tile
from concourse import bass_utils

F32 = mybir.dt.float32
BF16 = mybir.dt.bfloat16
AF = mybir.ActivationFunctionType

N_CORES = 8
B, S, HID = 1, 3072, 1024
NH, HD = 16, 64
PD = 128                   # pair dims per core (2 heads x 64)
QC = 512                   # query chunk (moving cols per matmul)
NQC = S // QC              # 6 query chunks
KT = S // 128              # 24 key tiles
NG = KT // 3               # 8 groups of 3 key tiles for batched exp
VTW = 193                  # per-kt v tile: vE(64)|1|vO(64)|1|zero-pad(63)

_cache: dict = {}


def _build(with_mask: bool, with_bias: bool):
    nc = bacc.Bacc("TRN2", target_bir_lowering=False, debug=False,
                   num_devices=N_CORES)

    J = 9 if with_bias else 8          # contraction slices (128 rows each)
    KIN = HID + 1 if with_bias else HID

    xt = nc.dram_tensor("xt", [KIN, S], BF16, kind="ExternalInput")
    w = nc.dram_tensor("w", [3, KIN, PD], BF16, kind="ExternalInput")
    ident = nc.dram_tensor("ident", [128, 128], F32, kind="ExternalInput")
    identb = nc.dram_tensor("identb", [128, 128], BF16, kind="ExternalInput")
    if with_mask:
        maskt = nc.dram_tensor("maskt", [128, KT], F32, kind="ExternalInput")
    out = nc.dram_tensor("out", [S, PD], F32, kind="ExternalOutput")

    with tile.TileContext(nc) as tc:
        with tc.tile_pool(name="persist", bufs=1) as pp:
            # ---- persistent SBUF tensors ----
            xsb = pp.tile([128, J * S], BF16)        # x^T contraction slices
            qz = pp.tile([128, 2 * S], BF16)         # [qE;0] | [0;qO]
            ksb = pp.tile([128, S], BF16)            # k^T pair block
            vtmp = pp.tile([128, S], BF16)           # v^T before transpose
            vsb = [pp.tile([128, VTW], BF16, name=f"vsb{k}") for k in range(KT)]
            idsb = pp.tile([128, 128], F32)
            idbsb = pp.tile([128, 128], BF16)
            ctxsb = [pp.tile([65, S], F32, name=f"ctxsb{h}") for h in range(2)]
            osb = [pp.tile([128, PD], F32, name=f"osb{t}") for t in range(KT)]
            if with_mask:
                msb = pp.tile([128, KT], F32)
                nc.sync.dma_start(msb[:], maskt[:])

            # x^T load: J full 128-row slices (+ the ones row when biased)
            for j in range(8):
                nc.sync.dma_start(xsb[:, j * S:(j + 1) * S],
                                  xt[j * 128:(j + 1) * 128, :])
            if with_bias:
                nc.sync.dma_start(xsb[0:1, 8 * S:9 * S], xt[1024:1025, :])

            nc.vector.memset(qz[:], 0.0)             # zero-pad halves
            nc.sync.dma_start(idsb[:], ident[:])
            nc.sync.dma_start(idbsb[:], identb[:])
            for k in range(KT):
                nc.vector.memset(vsb[k][:, 130:VTW], 0.0)
                nc.vector.memset(vsb[k][:, 64:65], 1.0)
                nc.vector.memset(vsb[k][:, 129:130], 1.0)

            # ---- phase A: projections (pair dims only, full sequence) ----
            with (
                tc.tile_pool(name="wpool", bufs=1) as wpool,
                tc.tile_pool(name="ppsum", bufs=4, space="PSUM") as ppsum,
                tc.tile_pool(name="tpsum", bufs=2, space="PSUM") as tpsum,
            ):
                def load_w(proj):
                    wt = []
                    for j in range(J):
                        if j < 8:
                            t = wpool.tile([128, PD], BF16, tag="w", bufs=24,
                                           name=f"w{proj}_{j}")
                            nc.sync.dma_start(t[:], w[proj, j * 128:(j + 1) * 128, :])
                        else:
                            t = wpool.tile([1, PD], BF16, tag="wb", bufs=3,
                                           name=f"wb{proj}")
                            nc.sync.dma_start(t[:], w[proj, HID:HID + 1, :])
                        wt.append(t)
                    return wt

                wq, wk, wv = load_w(0), load_w(1), load_w(2)

                def proj(wt, m, nm):
                    # one 512-col chunk of this projection's [128, S] output
                    pt = ppsum.tile([128, QC], F32, tag="pj", name=f"pj{nm}{m}")
                    for j in range(J):
                        rows = 128 if j < 8 else 1
                        nc.tensor.matmul(
                            pt[:],
                            wt[j][:rows, :],
                            xsb[:rows, j * S + m * QC: j * S + (m + 1) * QC],
                            start=(j == 0), stop=(j == J - 1))
                    return pt

                for m in range(NQC):
                    pk = proj(wk, m, "k")
                    nc.vector.tensor_copy(ksb[:, m * QC:(m + 1) * QC], pk[:])
                for m in range(NQC):
                    pv = proj(wv, m, "v")
                    nc.vector.tensor_copy(vtmp[:, m * QC:(m + 1) * QC], pv[:])
                    # transpose this chunk's 4 key-blocks into [key, dim] tiles
                    for b in range(4):
                        kt = m * 4 + b
                        tpv = tpsum.tile([128, 128], BF16, tag="tpv",
                                         name=f"tpv{kt}")
                        nc.tensor.transpose(
                            tpv[:], vtmp[:, kt * 128:(kt + 1) * 128], idbsb[:])
                        nc.vector.tensor_copy(vsb[kt][:, 0:64], tpv[:, 0:64])
                        nc.vector.tensor_copy(vsb[kt][:, 65:129], tpv[:, 64:128])
                for m in range(NQC):
                    pq = proj(wq, m, "q")
                    # zero-padded halves: head-even rows 0-63 in cols 0:S,
                    # head-odd rows 64-127 in cols S:2S
                    nc.vector.tensor_copy(qz[0:64, m * QC:(m + 1) * QC],
                                          pq[0:64, :])
                    nc.vector.tensor_copy(qz[64:128, S + m * QC:S + (m + 1) * QC],
                                          pq[64:128, :])

            # ---- phase C: attention, staggered even/odd heads ----
            with (
                tc.tile_pool(name="spoolE", bufs=1, space="PSUM") as spoolE,
                tc.tile_pool(name="spoolO", bufs=1, space="PSUM") as spoolO,
                tc.tile_pool(name="cpool", bufs=1, space="PSUM") as cpool,
                tc.tile_pool(name="ppool", bufs=10) as ppool,
            ):
                def score_block(sp, e, qc, g):
                    for j in range(3):
                        kt = g * 3 + j
                        nc.tensor.matmul(
                            sp[:, j * QC:(j + 1) * QC],
                            ksb[:, kt * 128:(kt + 1) * 128],
                            qz[:, e * S + qc * QC:e * S + (qc + 1) * QC],
                            start=True, stop=True)

                def exp_block(pt, sp, g):
                    if with_mask:
                        for j in range(3):
                            kt = g * 3 + j
                            nc.scalar.activation(
                                pt[:, j * QC:(j + 1) * QC],
                                sp[:, j * QC:(j + 1) * QC], AF.Exp,
                                bias=msb[:, kt:kt + 1], scale=0.125)
                    else:
                        nc.scalar.activation(pt[:], sp[:], AF.Exp, scale=0.125)

                def pv_block(ctx, pt, off, g):
                    # stationary = [vE|1|vO|1|pad] window at col offset 0/65;
                    # out rows 0-63 = ctx, row 64 = denominator, rows 65-127
                    # accumulate the other head's garbage, never read.
                    for j in range(3):
                        kt = g * 3 + j
                        nc.tensor.matmul(
                            ctx[:], vsb[kt][:, off:off + 128],
                            pt[:, j * QC:(j + 1) * QC],
                            start=(g == 0 and j == 0),
                            stop=(g == NG - 1 and j == 2))

                for qc in range(NQC):
                    ctxE = cpool.tile([128, QC], F32, tag="ctxE", name=f"cE{qc}")
                    ctxO = cpool.tile([128, QC], F32, tag="ctxO", name=f"cO{qc}")
                    for g in range(NG):
                        spE = spoolE.tile([128, 3 * QC], F32, tag="spE",
                                          name=f"spE{qc}_{g}")
                        score_block(spE, 0, qc, g)
                        ptE = ppool.tile([128, 3 * QC], BF16, tag="pt",
                                         name=f"ptE{qc}_{g}")
                        exp_block(ptE, spE, g)
                        spO = spoolO.tile([128, 3 * QC], F32, tag="spO",
                                          name=f"spO{qc}_{g}")
                        score_block(spO, 1, qc, g)
                        ptO = ppool.tile([128, 3 * QC], BF16, tag="pt",
                                         name=f"ptO{qc}_{g}")
                        exp_block(ptO, spO, g)
                        pv_block(ctxE, ptE, 0, g)
                        pv_block(ctxO, ptO, 65, g)
                    nc.vector.tensor_copy(ctxsb[0][:, qc * QC:(qc + 1) * QC],
                                          ctxE[0:65, :])
                    nc.vector.tensor_copy(ctxsb[1][:, qc * QC:(qc + 1) * QC],
                                          ctxO[0:65, :])

            # ---- phase D: transpose back, normalize, store ----
            with (
                tc.tile_pool(name="tpool", bufs=8, space="PSUM") as tpool,
                tc.tile_pool(name="rpool2", bufs=8) as rpool2,
            ):
                for h in range(2):
                    for t in range(KT):
                        tp = tpool.tile([128, 65], F32, tag="tp",
                                        name=f"tp{h}_{t}")
                        nc.tensor.transpose(
                            tp[:], ctxsb[h][:, t * 128:(t + 1) * 128],
                            idsb[0:65, 0:65])
                        rec = rpool2.tile([128, 1], F32, tag="rec",
                                          name=f"rec{h}_{t}")
                        nc.vector.reciprocal(rec[:], tp[:, 64:65])
                        nc.vector.tensor_scalar_mul(
                            osb[t][:, h * HD:(h + 1) * HD], tp[:, 0:64], rec[:])
                for t in range(KT):
                    nc.sync.dma_start(out[t * 128:(t + 1) * 128, :], osb[t][:])

    nc.compile()
    return nc


def _get_program(with_mask: bool, with_bias: bool):
    key = ("prog", with_mask, with_bias)
    if key not in _cache:
        _cache[key] = _build(with_mask, with_bias)
    return _cache[key]


def kernel(hidden_states, attention_mask, Wq, bq, Wk, bk, Wv, bv):
    x = np.asarray(hidden_states, np.float32).reshape(S, HID)
    mask = np.asarray(attention_mask, np.float32).reshape(-1)
    if mask.size == 1:
        mask = np.full(S, float(mask[0]), np.float32)
    with_mask = bool(np.any(mask))
    with_bias = bool(np.any(np.asarray(bq)) or np.any(np.asarray(bk))
                     or np.any(np.asarray(bv)))

    KIN = HID + 1 if with_bias else HID
    xtc = np.empty((KIN, S), np.float32)
    xtc[:HID] = x.T
    if with_bias:
        xtc[HID] = 1.0
    xtc = xtc.astype(ml_dtypes.bfloat16)

    # augmented weights: [3, KIN, 1024] with the bias as the last
    # contraction row; per-core slice is its pair's 128 output dims.
    w_aug = np.empty((3, KIN, HID), np.float32)
    for i, (W, b) in enumerate(((Wq, bq), (Wk, bk), (Wv, bv))):
        w_aug[i, :HID] = np.asarray(W, np.float32).T
        if with_bias:
            w_aug[i, HID] = np.asarray(b, np.float32)
    w_aug = w_aug.astype(ml_dtypes.bfloat16)

    ident = np.eye(128, dtype=np.float32)
    identb = np.eye(128, dtype=ml_dtypes.bfloat16)

    nc = _get_program(with_mask, with_bias)
    in_maps = []
    for c in range(N_CORES):
        m = {
            "xt": xtc,
            "w": np.ascontiguousarray(w_aug[:, :, c * PD:(c + 1) * PD]),
            "ident": ident,
            "identb": identb,
        }
        if with_mask:
            m["maskt"] = np.ascontiguousarray(
                mask.reshape(KT, 128).T.astype(np.float32))
        in_maps.append(m)

    _cache["last_in_maps"] = in_maps
    _cache["last_prog"] = nc
    res = bass_utils.run_bass_kernel_spmd(nc, in_maps, core_ids=list(range(N_CORES)))
    out = np.concatenate([res.results[c]["out"] for c in range(N_CORES)], axis=1)
    return out.reshape(B, S, HID).astype(np.float32)
